# revision 1
# baseline (speedup 1.0000x reference)
"""Trainium2 Bass kernel for BertAlibiUnpadSelfAttention.

Problem shapes (hardcoded): B=2, S=2048, H=12, D=64, DIM=768.
Reference computation:
    qkv = hidden @ Wqkv_w.T + Wqkv_b            # (4096, 2304)
    pad via indices (a permutation -> pure row shuffle)
    q,k,v = split/reshape -> (b, h, s, d)
    scores = q @ k.T / sqrt(64) + bias          # bias dense (2,12,2048,2048)
    attn = softmax(scores) @ v -> (4096, 768), unpad via indices

Sharding: 24 (batch, head) pairs -> 3 per core across 8 cores. Each core
computes its own slice of the QKV projection (disjoint columns/rows -> no
redundant FLOPs) and full attention for its 3 heads.

Device kernel layout choices:
  - qT/kT computed in [d, s] layout directly (lhsT = W slices, rhs = hidden^T),
    which is exactly the layout the scores matmul wants.  1/sqrt(D) folded
    into Wq + bq on the host.
  - scores are computed TRANSPOSED: scoresT[sk, sq] tiles, so the softmax
    reduction (over sk) can be done by the PV matmul itself: V gets an
    appended ones-column, so PV produces [attnT ; sums] in one accumulation.
  - bias is pre-transposed per head on the host; VectorE adds it to the
    score PSUM; ScalarE applies exp (no max subtraction: logits ~ N(0,1),
    fp32 exp is exact-safe here).
  - Final normalize (divide by sums) + transpose back to [s, d] + V-bias add
    happen on the host (tiny: 3x65x2048 per core).
"""

import math
import numpy as np

B, S, H, D = 2, 2048, 12, 64
DIM = H * D            # 768
TOTAL = B * S          # 4096
HPC = 3                # heads per core
N_CORES = 8
KT = DIM // 128        # 6 k-tiles of 128
SQC = S // 512         # 4 free-dim chunks of 512
SKT = S // 128         # 16 sk tiles of 128

_CACHE = {}


def _build_nc(variant="inject"):
    """Build + compile the per-core Bass module.

    All matmuls use tf32 (float32r) operands, fp32 PSUM accumulation.
    The dense additive bias is shipped as fp16 (error ~5e-5, halves DMA).

    variant:
      "inject" - bias tiles are injected into PSUM via an fp16 identity
                 matmul before the QK accumulation; ScalarE exp reads the
                 biased scores straight from PSUM.  Keeps the PE stream
                 dense (HAM-friendly) and VectorE nearly idle.
      "expb"   - host ships exp(bias) instead; scores exp'd from PSUM and
                 multiplied by expb on VectorE.  Fewer PE instructions.
    """
    from concourse import bacc, mybir, tile

    f32 = mybir.dt.float32
    f16 = mybir.dt.float16
    proj_dt = f16
    att_dt = f16

    def mmap(ap):
        return ap

    nc = bacc.Bacc("TRN2", target_bir_lowering=False, debug=False)

    hT = nc.dram_tensor("hT", (DIM, S), proj_dt, kind="ExternalInput")
    wq = nc.dram_tensor("wq", (DIM, HPC * D), proj_dt, kind="ExternalInput")
    wk = nc.dram_tensor("wk", (DIM, HPC * D), proj_dt, kind="ExternalInput")
    wv = nc.dram_tensor("wv", (DIM, HPC * D), proj_dt, kind="ExternalInput")
    bq = nc.dram_tensor("bq", (HPC * D, 1), f32, kind="ExternalInput")
    bk = nc.dram_tensor("bk", (HPC * D, 1), f32, kind="ExternalInput")
    bias_t = nc.dram_tensor("bias_t", (HPC, S, S), f16, kind="ExternalInput")
    ident = nc.dram_tensor("ident", (128, 128), f16, kind="ExternalInput")
    out = nc.dram_tensor("out", (HPC, D + 1, S), f32, kind="ExternalOutput")

    EXP = mybir.ActivationFunctionType.Exp

    with tile.TileContext(nc) as tc:
        with (
            tc.tile_pool(name="const", bufs=1) as constp,
            tc.tile_pool(name="bias", bufs=10) as biasp,
            tc.tile_pool(name="pt", bufs=8) as ptp,
            tc.tile_pool(name="ot", bufs=3) as otp,
        ):
            # ---- load persistent inputs (small tensors first, on the
            # scalar HWDGE queue; hT on the sync queue) ----
            ht = [constp.tile([128, S], proj_dt, tag=f"ht{i}", name=f"ht{i}") for i in range(KT)]
            for i in range(KT):
                nc.sync.dma_start(ht[i][:], hT[i * 128:(i + 1) * 128, :])

            wq_sb = [constp.tile([128, HPC * D], proj_dt, tag=f"wq{i}", name=f"wq{i}") for i in range(KT)]
            wk_sb = [constp.tile([128, HPC * D], proj_dt, tag=f"wk{i}", name=f"wk{i}") for i in range(KT)]
            wv_sb = [constp.tile([128, HPC * D], proj_dt, tag=f"wv{i}", name=f"wv{i}") for i in range(KT)]
            bq_sb = constp.tile([128, 1], f32, tag="bq0")
            bq_sb2 = constp.tile([64, 1], f32, tag="bq1")
            bk_sb = constp.tile([128, 1], f32, tag="bk0")
            bk_sb2 = constp.tile([64, 1], f32, tag="bk1")
            ident_sb = constp.tile([128, 128], f16, tag="ident")
            nc.scalar.dma_start(ident_sb[:], ident[:, :])
            nc.scalar.dma_start(bq_sb[:], bq[0:128, :])
            nc.scalar.dma_start(bq_sb2[:], bq[128:192, :])
            nc.scalar.dma_start(bk_sb[:], bk[0:128, :])
            nc.scalar.dma_start(bk_sb2[:], bk[128:192, :])
            for i in range(KT):
                nc.scalar.dma_start(wq_sb[i][:], wq[i * 128:(i + 1) * 128, :])
                nc.scalar.dma_start(wk_sb[i][:], wk[i * 128:(i + 1) * 128, :])
                nc.scalar.dma_start(wv_sb[i][:], wv[i * 128:(i + 1) * 128, :])
            # Q/K in [d, s] layout: heads 0,1 in tile0 (partitions 0-63 /
            # 64-127), head 2 in tile1 (partitions 0-63).  Same base
            # partition for q_j and k_j so the scores matmul operands align.
            q0 = constp.tile([128, S], att_dt, tag="q0")
            q1 = constp.tile([64, S], att_dt, tag="q1")
            k0 = constp.tile([128, S], att_dt, tag="k0")
            k1 = constp.tile([64, S], att_dt, tag="k1")
            # V' per head: [sk, 65] blocks stacked along free dim; col 64
            # stays 1.0 so PV also produces the softmax row-sums.
            vp = [constp.tile([128, SKT * (D + 1)], att_dt, tag=f"vp{j}", name=f"vp{j}")
                  for j in range(HPC)]
            for j in range(HPC):
                nc.vector.memset(vp[j][:], 1.0)

            # ---- phase 1a: qT / kT projection (+ bias, per-partition) ----
            with tc.tile_pool(name="psA", bufs=2, space="PSUM") as psA:
                for (dst, wsb, bsb, col0, m) in (
                    (q0, wq_sb, bq_sb, 0, 128),
                    (q1, wq_sb, bq_sb2, 128, 64),
                    (k0, wk_sb, bk_sb, 0, 128),
                    (k1, wk_sb, bk_sb2, 128, 64),
                ):
                    for c in range(SQC):
                        ps = psA.tile([m, 512], f32, tag=f"psA{m}", name=f"psA{m}")
                        for i in range(KT):
                            nc.tensor.matmul(
                                ps[:],
                                mmap(wsb[i][:, col0:col0 + m]),
                                mmap(ht[i][:, c * 512:(c + 1) * 512]),
                                start=(i == 0), stop=(i == KT - 1),
                            )
                        nc.vector.tensor_scalar_add(
                            dst[:, c * 512:(c + 1) * 512], ps[:], bsb[:])

                # ---- phase 1b: V in natural [s, d] layout ----
                for st in range(SKT):
                    psv = psA.tile([128, HPC * D], f32, tag="psV", name="psV")
                    for i in range(KT):
                        nc.tensor.matmul(
                            psv[:],
                            mmap(ht[i][:, st * 128:(st + 1) * 128]),
                            mmap(wv_sb[i][:]),
                            start=(i == 0), stop=(i == KT - 1),
                        )
                    for j in range(HPC):
                        nc.vector.tensor_copy(
                            vp[j][:, st * (D + 1):st * (D + 1) + D],
                            psv[:, j * D:(j + 1) * D])

            # ---- phase 2: attention per head ----
            qk_slices = (  # (q_ap, k_ap) per head, matching base partitions
                (q0[0:64, :], k0[0:64, :]),
                (q0[64:128, :], k0[64:128, :]),
                (q1[:, :], k1[:, :]),
            )
            with (
                tc.tile_pool(name="ps", bufs=2, space="PSUM") as psp,
                tc.tile_pool(name="po", bufs=4, space="PSUM") as pop,
            ):
                for j in range(HPC):
                    qap, kap = qk_slices[j]
                    po = [pop.tile([D + 1, 512], f32, tag="po", name=f"po{j}_{_c}") for _c in range(SQC)]
                    for st in range(SKT):
                        bt = biasp.tile([128, S], f16, name="bt")
                        dma_eng = (nc.sync, nc.scalar)[(j * SKT + st) % 2]
                        dma_eng.dma_start(
                            bt[:], bias_t[j, st * 128:(st + 1) * 128, :])
                        for half in range(2):
                            ps = psp.tile([128, 1024], f32, name="ps")
                            pt = ptp.tile([128, 1024], att_dt, name="pt")
                            for cc in range(2):
                                c = half * 2 + cc
                                sq = slice(c * 512, (c + 1) * 512)
                                if variant == "inject":
                                    nc.tensor.matmul(
                                        ps[:, cc * 512:(cc + 1) * 512],
                                        ident_sb[:],
                                        bt[:, sq],
                                        start=True, stop=False,
                                    )
                                nc.tensor.matmul(
                                    ps[:, cc * 512:(cc + 1) * 512],
                                    kap[:, st * 128:(st + 1) * 128],
                                    qap[:, sq],
                                    start=(variant != "inject"),
                                    stop=True,
                                )
                            if variant == "inject":
                                nc.scalar.activation(pt[:], ps[:], EXP)
                            else:
                                nc.scalar.activation(pt[:], ps[:], EXP)
                                nc.vector.tensor_mul(
                                    pt[:], pt[:],
                                    bt[:, half * 1024:(half + 1) * 1024])
                            for cc in range(2):
                                c = half * 2 + cc
                                nc.tensor.matmul(
                                    po[c][:],
                                    mmap(vp[j][:, st * (D + 1):(st + 1) * (D + 1)]),
                                    pt[:, cc * 512:(cc + 1) * 512],
                                    start=(st == 0), stop=(st == SKT - 1),
                                )
                    for c in range(SQC):
                        ot = otp.tile([D + 1, 512], f32, name="ot")
                        nc.vector.tensor_copy(ot[:], po[c][:])
                        nc.sync.dma_start(
                            out[j, :, c * 512:(c + 1) * 512], ot[:])

    nc.compile()
    return nc


def _get_nc(variant="inject"):
    if variant not in _CACHE:
        _CACHE[variant] = _build_nc(variant)
    return _CACHE[variant]


def _make_in_maps(hidden_states, Wqkv_w, Wqkv_b, bias, indices, variant="inject"):
    hidden_states = np.asarray(hidden_states, dtype=np.float32)
    Wqkv_w = np.asarray(Wqkv_w, dtype=np.float32)
    Wqkv_b = np.asarray(Wqkv_b, dtype=np.float32)
    bias = np.asarray(bias, dtype=np.float32)
    indices = np.asarray(indices, dtype=np.int64)

    scale = 1.0 / math.sqrt(D)
    padded = np.zeros((TOTAL, DIM), dtype=np.float32)
    padded[indices] = hidden_states

    Wq, Wk, Wv = Wqkv_w[0:DIM], Wqkv_w[DIM:2 * DIM], Wqkv_w[2 * DIM:3 * DIM]
    bq_full = Wqkv_b[0:DIM] * scale
    bk_full = Wqkv_b[DIM:2 * DIM]
    ident = np.eye(128, dtype=np.float16)

    in_maps = []
    for c in range(N_CORES):
        b = c // 4
        h0 = (c % 4) * HPC
        r = slice(h0 * D, (h0 + HPC) * D)
        bias_c = bias[b, h0:h0 + HPC].transpose(0, 2, 1)
        if variant == "expb":
            bias_c = np.exp(bias_c)
        in_maps.append({
            "hT": padded[b * S:(b + 1) * S].T.astype(np.float16),
            "wq": (Wq[r].T * np.float32(scale)).astype(np.float16),
            "wk": Wk[r].T.astype(np.float16),
            "wv": Wv[r].T.astype(np.float16),
            "bq": np.ascontiguousarray(bq_full[r].reshape(HPC * D, 1)),
            "bk": np.ascontiguousarray(bk_full[r].reshape(HPC * D, 1)),
            "bias_t": np.ascontiguousarray(bias_c.astype(np.float16)),
            "ident": ident,
        })
    return in_maps


def _assemble(results, Wqkv_b, indices):
    Wqkv_b = np.asarray(Wqkv_b, dtype=np.float32)
    indices = np.asarray(indices, dtype=np.int64)
    bv = Wqkv_b[2 * DIM:3 * DIM]
    out_full = np.empty((TOTAL, DIM), dtype=np.float32)
    for c in range(N_CORES):
        b = c // 4
        h0 = (c % 4) * HPC
        o = np.asarray(results[c]["out"], dtype=np.float32)  # (3, 65, 2048)
        for j in range(HPC):
            h = h0 + j
            att = (o[j, :D] / o[j, D]).T + bv[h * D:(h + 1) * D]
            out_full[b * S:(b + 1) * S, h * D:(h + 1) * D] = att
    return out_full[indices]


VARIANT = "expb"


def kernel(hidden_states, Wqkv_w, Wqkv_b, bias, slopes, cu_seqlens, indices,
           attn_mask, max_seqlen, **_unused):
    from concourse.bass_utils import run_bass_kernel_spmd

    nc = _get_nc(VARIANT)
    in_maps = _make_in_maps(hidden_states, Wqkv_w, Wqkv_b, bias, indices,
                            VARIANT)
    res = run_bass_kernel_spmd(nc, in_maps, list(range(N_CORES)))
    return _assemble(res.results, Wqkv_b, indices)



# revision 2
# speedup vs baseline: 1.0192x; 1.0192x over previous
"""Trainium2 Bass kernel for BertAlibiUnpadSelfAttention.

Problem shapes (hardcoded): B=2, S=2048, H=12, D=64, DIM=768.
Reference computation:
    qkv = hidden @ Wqkv_w.T + Wqkv_b            # (4096, 2304)
    pad via indices (a permutation -> pure row shuffle)
    q,k,v = split/reshape -> (b, h, s, d)
    scores = q @ k.T / sqrt(64) + bias          # bias dense (2,12,2048,2048)
    attn = softmax(scores) @ v -> (4096, 768), unpad via indices

Sharding: 24 (batch, head) pairs -> 3 per core across 8 cores. Each core
computes its own slice of the QKV projection (disjoint columns/rows -> no
redundant FLOPs) and full attention for its 3 heads.

Device kernel layout choices (v2 - engine-balanced softmax):
  - qT/kT computed in [d, s] layout directly (lhsT = W slices, rhs = hidden^T),
    which is exactly the layout the scores matmul wants.  The Q side is
    pre-scaled by A/sqrt(D) with A = 2^10/ln2, so PSUM scores are s*A.
  - scores are computed TRANSPOSED: scoresT[sk, sq] tiles, so the softmax
    reduction (over sk) can be done by the PV matmul itself: V gets an
    appended ones-column, so PV produces [attnT ; sums] in one accumulation.
  - exp() is SPLIT across two engines by query half to balance load:
      half 0 (path A): ScalarE ACTIVATE Exp (scale=1/A) -> fp16, then
        VectorE multiply by exp(bias) shipped as fp16 (2x DVE mode).
      half 1 (path B): single VectorE tensor_tensor: int16(round(s*A + b*A
        + 15360 - C)) whose BITS are the fp16 Schraudolph approximation of
        exp(s+b) (max +-4% sawtooth, zero mean log error with C=59.65;
        averages out to ~0.1% after the PV reduction).
    Since the A/B split is by query (softmax rows are per-query over all
    keys), each softmax row is either pure-exact or pure-Schraudolph, and
    the Schraudolph systematic factor cancels in the softmax normalization.
  - Projection/V/output PSUM evacuations run on ScalarE (ACTIVATE
    Identity with per-partition bias / Copy) to keep VectorE free for the
    softmax tensor_tensor work.
  - Final normalize (divide by sums) + transpose back to [s, d] + V-bias add
    happen on the host (tiny: 3x65x2048 per core).
"""

import math
import numpy as np

B, S, H, D = 2, 2048, 12, 64
DIM = H * D            # 768
TOTAL = B * S          # 4096
HPC = 3                # heads per core
N_CORES = 8
KT = DIM // 128        # 6 k-tiles of 128
SQC = S // 512         # 4 free-dim chunks of 512
SKT = S // 128         # 16 sk tiles of 128
VST = HPC * 65         # vp cols per st block: [h0 64 + one | h1 ... | h2 ...]

A_EXP = 1024.0 / math.log(2.0)   # fp16 Schraudolph scale, 1477.32
C_OPT = 59.65                    # zero-mean-log correction
B_OFF = 15360.0 - C_OPT          # 15*1024 - C

_CACHE = {}


def _build_nc():
    """Build + compile the per-core Bass module (fp16 operands, fp32 PSUM)."""
    from concourse import bacc, mybir, tile

    f32 = mybir.dt.float32
    f16 = mybir.dt.float16
    i16 = mybir.dt.int16

    nc = bacc.Bacc("TRN2", target_bir_lowering=False, debug=False)

    hT = nc.dram_tensor("hT", (DIM, S), f16, kind="ExternalInput")
    wq = nc.dram_tensor("wq", (DIM, HPC * D), f16, kind="ExternalInput")
    wk = nc.dram_tensor("wk", (DIM, HPC * D), f16, kind="ExternalInput")
    wv = nc.dram_tensor("wv", (DIM, HPC * D), f16, kind="ExternalInput")
    bq = nc.dram_tensor("bq", (HPC * D, 1), f32, kind="ExternalInput")
    bk = nc.dram_tensor("bk", (HPC * D, 1), f32, kind="ExternalInput")
    # per head [sk, sq]; cols 0:1024 fp16 exp(bias) bits, cols 1024:2048
    # int16 round(bias*A + B_OFF)
    bias_t = nc.dram_tensor("bias_t", (HPC, S, S), i16, kind="ExternalInput")
    out = nc.dram_tensor("out", (HPC, D + 1, S), f32, kind="ExternalOutput")

    EXP = mybir.ActivationFunctionType.Exp
    IDENT = mybir.ActivationFunctionType.Identity
    ADD = mybir.AluOpType.add

    with tile.TileContext(nc) as tc:
        with (
            tc.tile_pool(name="const", bufs=1) as constp,
            tc.tile_pool(name="bias", bufs=6) as biasp,
            tc.tile_pool(name="pt", bufs=6) as ptp,
            tc.tile_pool(name="ot", bufs=3) as otp,
        ):
            # ---- load persistent inputs ----
            ht = [constp.tile([128, S], f16, tag=f"ht{i}", name=f"ht{i}") for i in range(KT)]
            for i in range(KT):
                nc.sync.dma_start(ht[i][:], hT[i * 128:(i + 1) * 128, :])

            wq_sb = [constp.tile([128, HPC * D], f16, tag=f"wq{i}", name=f"wq{i}") for i in range(KT)]
            wk_sb = [constp.tile([128, HPC * D], f16, tag=f"wk{i}", name=f"wk{i}") for i in range(KT)]
            wv_sb = [constp.tile([128, HPC * D], f16, tag=f"wv{i}", name=f"wv{i}") for i in range(KT)]
            bq_sb = constp.tile([128, 1], f32, tag="bq0")
            bq_sb2 = constp.tile([64, 1], f32, tag="bq1")
            bk_sb = constp.tile([128, 1], f32, tag="bk0")
            bk_sb2 = constp.tile([64, 1], f32, tag="bk1")
            nc.scalar.dma_start(bq_sb[:], bq[0:128, :])
            nc.scalar.dma_start(bq_sb2[:], bq[128:192, :])
            nc.scalar.dma_start(bk_sb[:], bk[0:128, :])
            nc.scalar.dma_start(bk_sb2[:], bk[128:192, :])
            for i in range(KT):
                nc.scalar.dma_start(wq_sb[i][:], wq[i * 128:(i + 1) * 128, :])
                nc.scalar.dma_start(wk_sb[i][:], wk[i * 128:(i + 1) * 128, :])
                nc.scalar.dma_start(wv_sb[i][:], wv[i * 128:(i + 1) * 128, :])
            # Q/K in [d, s] layout: heads 0,1 in tile0 (partitions 0-63 /
            # 64-127), head 2 in tile1 (partitions 0-63).  Same base
            # partition for q_j and k_j so the scores matmul operands align.
            q0 = constp.tile([128, S], f16, tag="q0")
            q1 = constp.tile([64, S], f16, tag="q1")
            k0 = constp.tile([128, S], f16, tag="k0")
            k1 = constp.tile([64, S], f16, tag="k1")
            # V' blocks per st: [h0 d0..63, one, h1 d0..63, one, h2 ...];
            # the ones come from the memset and give the softmax row-sums.
            vp = constp.tile([128, SKT * VST], f16, tag="vp")
            nc.vector.memset(vp[:], 1.0)

            # ---- phase 1a: qT / kT projection (+ bias via ScalarE) ----
            with tc.tile_pool(name="psA", bufs=2, space="PSUM") as psA:
                for (dst, wsb, bsb, col0, m) in (
                    (q0, wq_sb, bq_sb, 0, 128),
                    (q1, wq_sb, bq_sb2, 128, 64),
                    (k0, wk_sb, bk_sb, 0, 128),
                    (k1, wk_sb, bk_sb2, 128, 64),
                ):
                    for c in range(SQC):
                        ps = psA.tile([m, 512], f32, tag=f"psA{m}", name=f"psA{m}")
                        for i in range(KT):
                            nc.tensor.matmul(
                                ps[:],
                                wsb[i][:, col0:col0 + m],
                                ht[i][:, c * 512:(c + 1) * 512],
                                start=(i == 0), stop=(i == KT - 1),
                            )
                        nc.scalar.activation(
                            dst[:, c * 512:(c + 1) * 512], ps[:], IDENT,
                            bias=bsb[:])

                # ---- phase 1b: V in natural [s, d] layout ----
                for st in range(SKT):
                    psv = psA.tile([128, HPC * D], f32, tag="psV", name="psV")
                    for i in range(KT):
                        nc.tensor.matmul(
                            psv[:],
                            ht[i][:, st * 128:(st + 1) * 128],
                            wv_sb[i][:],
                            start=(i == 0), stop=(i == KT - 1),
                        )
                    for j in range(HPC):
                        nc.scalar.copy(
                            vp[:, st * VST + j * 65: st * VST + j * 65 + D],
                            psv[:, j * D:(j + 1) * D])

            # ---- phase 2: attention per head ----
            qk_slices = (  # (q_ap, k_ap) per head, matching base partitions
                (q0[0:64, :], k0[0:64, :]),
                (q0[64:128, :], k0[64:128, :]),
                (q1[:, :], k1[:, :]),
            )
            with (
                tc.tile_pool(name="ps", bufs=2, space="PSUM") as psp,
                tc.tile_pool(name="po", bufs=4, space="PSUM") as pop,
            ):
                for j in range(HPC):
                    qap, kap = qk_slices[j]
                    po = [pop.tile([D + 1, 512], f32, tag="po", name=f"po{j}_{_c}") for _c in range(SQC)]
                    for st in range(SKT):
                        bt = biasp.tile([128, S], i16, name="bt")
                        dma_eng = (nc.sync, nc.scalar)[(j * SKT + st) % 2]
                        dma_eng.dma_start(
                            bt[:], bias_t[j, st * 128:(st + 1) * 128, :])
                        for half in range(2):
                            ps = psp.tile([128, 1024], f32, name="ps")
                            pt = ptp.tile([128, 1024], f16, name="pt")
                            for cc in range(2):
                                c = half * 2 + cc
                                sq = slice(c * 512, (c + 1) * 512)
                                nc.tensor.matmul(
                                    ps[:, cc * 512:(cc + 1) * 512],
                                    kap[:, st * 128:(st + 1) * 128],
                                    qap[:, sq],
                                    start=True, stop=True,
                                )
                            if half == 0:
                                # path A: exact exp on ScalarE, * exp(bias)
                                nc.scalar.activation(
                                    pt[:], ps[:], EXP, scale=1.0 / A_EXP)
                                nc.vector.tensor_mul(
                                    pt[:], pt[:],
                                    bt[:, 0:1024].bitcast(f16))
                            else:
                                # path B: Schraudolph exp via int16 convert
                                nc.vector.tensor_tensor(
                                    pt[:].bitcast(i16), ps[:],
                                    bt[:, 1024:2048], ADD)
                            for cc in range(2):
                                c = half * 2 + cc
                                nc.tensor.matmul(
                                    po[c][:],
                                    vp[:, st * VST + j * 65: st * VST + j * 65 + D + 1],
                                    pt[:, cc * 512:(cc + 1) * 512],
                                    start=(st == 0), stop=(st == SKT - 1),
                                )
                    for c in range(SQC):
                        ot = otp.tile([D + 1, 512], f32, name="ot")
                        nc.scalar.copy(ot[:], po[c][:])
                        nc.sync.dma_start(
                            out[j, :, c * 512:(c + 1) * 512], ot[:])

    nc.compile()
    return nc


def _get_nc(variant=None):
    if "nc" not in _CACHE:
        _CACHE["nc"] = _build_nc()
    return _CACHE["nc"]


def _make_in_maps(hidden_states, Wqkv_w, Wqkv_b, bias, indices, variant=None):
    hidden_states = np.asarray(hidden_states, dtype=np.float32)
    Wqkv_w = np.asarray(Wqkv_w, dtype=np.float32)
    Wqkv_b = np.asarray(Wqkv_b, dtype=np.float32)
    bias = np.asarray(bias, dtype=np.float32)
    indices = np.asarray(indices, dtype=np.int64)

    qscale = np.float32(A_EXP / math.sqrt(D))
    padded = np.zeros((TOTAL, DIM), dtype=np.float32)
    padded[indices] = hidden_states

    Wq, Wk, Wv = Wqkv_w[0:DIM], Wqkv_w[DIM:2 * DIM], Wqkv_w[2 * DIM:3 * DIM]
    bq_full = Wqkv_b[0:DIM] * qscale
    bk_full = Wqkv_b[DIM:2 * DIM]

    in_maps = []
    for cidx in range(N_CORES):
        b = cidx // 4
        h0 = (cidx % 4) * HPC
        r = slice(h0 * D, (h0 + HPC) * D)
        bias_c = np.ascontiguousarray(bias[b, h0:h0 + HPC].transpose(0, 2, 1))
        bt = np.empty((HPC, S, S), dtype=np.int16)
        bt[:, :, 0:1024] = np.exp(bias_c[:, :, 0:1024]).astype(np.float16).view(np.int16)
        bt[:, :, 1024:2048] = np.rint(
            bias_c[:, :, 1024:2048] * np.float32(A_EXP) + np.float32(B_OFF)
        ).astype(np.int16)
        in_maps.append({
            "hT": padded[b * S:(b + 1) * S].T.astype(np.float16),
            "wq": (Wq[r].T * qscale).astype(np.float16),
            "wk": Wk[r].T.astype(np.float16),
            "wv": Wv[r].T.astype(np.float16),
            "bq": np.ascontiguousarray(bq_full[r].reshape(HPC * D, 1)),
            "bk": np.ascontiguousarray(bk_full[r].reshape(HPC * D, 1)),
            "bias_t": bt,
        })
    return in_maps


def _assemble(results, Wqkv_b, indices):
    Wqkv_b = np.asarray(Wqkv_b, dtype=np.float32)
    indices = np.asarray(indices, dtype=np.int64)
    bv = Wqkv_b[2 * DIM:3 * DIM]
    out_full = np.empty((TOTAL, DIM), dtype=np.float32)
    for c in range(N_CORES):
        b = c // 4
        h0 = (c % 4) * HPC
        o = np.asarray(results[c]["out"], dtype=np.float32)  # (3, 65, 2048)
        for j in range(HPC):
            h = h0 + j
            att = (o[j, :D] / o[j, D]).T + bv[h * D:(h + 1) * D]
            out_full[b * S:(b + 1) * S, h * D:(h + 1) * D] = att
    return out_full[indices]


VARIANT = "v2"


def kernel(hidden_states, Wqkv_w, Wqkv_b, bias, slopes, cu_seqlens, indices,
           attn_mask, max_seqlen, **_unused):
    from concourse.bass_utils import run_bass_kernel_spmd

    nc = _get_nc()
    in_maps = _make_in_maps(hidden_states, Wqkv_w, Wqkv_b, bias, indices)
    res = run_bass_kernel_spmd(nc, in_maps, list(range(N_CORES)))
    return _assemble(res.results, Wqkv_b, indices)
